# revision 15
# baseline (speedup 1.0000x reference)
#
# DeepseekV4 Indexer kernel for 8x Trainium2 NeuronCores (Bass/Tile).
#
# Sharding: data-parallel over query-token tiles, interleaved so every core
# gets one tile from each causal-width band: core c owns global 128-row tiles
# {24+c, 16+c, 8+c, c} (slots 0..3). k is computed locally per-core for its
# own rows and all-gathered (transposed) across the 8 cores.
#
# Per (t,s) score tile: per head h, PE computes Z_h = q_hT.T @ kT (fp32),
# ACT evicts relu(Z_h) to SBUF, DVE accumulates acc += relu(Z_h) * w[t,h]
# via scalar_tensor_tensor (exact fp32, same head order as the reference
# scan). Causal masking is data-driven from the positions input so the
# program is SPMD-uniform. Top-k = iterative max8/max_index/match_replace
# extraction (exact, jax-stable tie order), with the deterministic
# "masked tail" of each row (output[t, r] = r for r > t) filled by a
# predicated iota overwrite instead of being extracted.
#
import sys
import math

sys.path.insert(0, "/opt/trn_rl_repo")

import numpy as np

T = 4096
HSZ = 7168
QLR = 1536
NH = 64
HD = 128
ROPE = 64
HALF = ROPE // 2  # 32
TOPK = 2048
NCORES = 8
P = 128

SLOT_BASE = [24, 16, 8, 0]          # slot s of core c handles global tile SLOT_BASE[s]+c
SLOT_W = [4096, 3072, 2048, 1024]   # uniform extraction/score width per slot
SLOT_NJ = [8, 6, 4, 2]              # number of 512-wide s-tiles computed per slot
SLOT_R = [256, 256, 256, 128]       # extraction rounds per slot
NSLOT = 4
TPC = NSLOT * P                      # tokens per core = 512

NEG_FILL = -3.0e38

_compiled = None
last_result = None


def _build():
    import concourse.bacc as bacc
    import concourse.mybir as mybir
    from concourse.tile import TileContext

    f32 = mybir.dt.float32
    u32 = mybir.dt.uint32
    i32 = mybir.dt.int32
    AF = mybir.ActivationFunctionType
    OP = mybir.AluOpType

    nc = bacc.Bacc("TRN2", target_bir_lowering=False, debug=False, num_devices=NCORES)

    # ---- per-core I/O ----
    qr_c = nc.dram_tensor("qr_c", [TPC, QLR], f32, kind="ExternalInput")
    hid_c = nc.dram_tensor("hid_c", [TPC, HSZ], f32, kind="ExternalInput")
    wq_b = nc.dram_tensor("wq_b", [QLR, NH * HD], f32, kind="ExternalInput")
    wk_in = nc.dram_tensor("wk_in", [HSZ, HD], f32, kind="ExternalInput")
    wp_in = nc.dram_tensor("wp_in", [HSZ, NH], f32, kind="ExternalInput")
    kg_in = nc.dram_tensor("kg_in", [1, HD], f32, kind="ExternalInput")
    kb_in = nc.dram_tensor("kb_in", [1, HD], f32, kind="ExternalInput")
    posc_in = nc.dram_tensor("posc_in", [TPC, 1], f32, kind="ExternalInput")
    spos_in = nc.dram_tensor("spos_in", [1, T], f32, kind="ExternalInput")
    cosk_in = nc.dram_tensor("cosk_in", [TPC, HALF], f32, kind="ExternalInput")
    sink_in = nc.dram_tensor("sink_in", [TPC, HALF], f32, kind="ExternalInput")
    # transposed + HD^-0.5-scaled rope tables for q (freq-major, core tokens)
    cosqT_in = nc.dram_tensor("cosqT_in", [HALF, TPC], f32, kind="ExternalInput")
    sinqT_in = nc.dram_tensor("sinqT_in", [HALF, TPC], f32, kind="ExternalInput")
    ident_in = nc.dram_tensor("ident_in", [P, P], f32, kind="ExternalInput")
    swap_in = nc.dram_tensor("swap_in", [P, ROPE], f32, kind="ExternalInput")
    iota_in = nc.dram_tensor("iota_in", [1, TOPK], u32, kind="ExternalInput")

    scores_c = nc.dram_tensor("scores_c", [TPC, T], f32, kind="ExternalOutput")
    idx_c = nc.dram_tensor("idx_c", [TPC, TOPK], i32, kind="ExternalOutput")

    # collectives buffers
    ktl = nc.dram_tensor("ktl", [NSLOT, P, P], f32)
    ktg = nc.dram_tensor("ktg", [NCORES, NSLOT, P, P], f32, addr_space="Shared")

    qscale = float(HD ** -0.5)
    wscale = float(NH ** -0.5)

    with TileContext(nc) as tc:
        with tc.tile_pool(name="const", bufs=1) as cp:
            ident = cp.tile([P, P], f32)
            nc.sync.dma_start(ident[:], ident_in[:])
            neginf = cp.tile([P, 512], f32)
            nc.vector.memset(neginf[:], float("-inf"))
            negfill = cp.tile([P, 512], f32)
            nc.vector.memset(negfill[:], NEG_FILL)
            iota_u = cp.tile([P, TOPK], u32)
            nc.sync.dma_start(iota_u[:], iota_in[:].partition_broadcast(P))
            iota_f = cp.tile([P, TOPK], f32)
            nc.vector.tensor_copy(iota_f[:], iota_u[:])
            spos = cp.tile([P, T], f32)
            nc.sync.dma_start(spos[:], spos_in[:].partition_broadcast(P))
            posc = cp.tile([P, NSLOT], f32)
            for s in range(NSLOT):
                nc.sync.dma_start(posc[:, s : s + 1], posc_in[s * P : (s + 1) * P, :])
            kgb = cp.tile([P, 2 * HD], f32)
            nc.sync.dma_start(kgb[:, :HD], kg_in[:].partition_broadcast(P))
            nc.sync.dma_start(kgb[:, HD:], kb_in[:].partition_broadcast(P))
            cosk = cp.tile([P, NSLOT * HALF], f32)
            sink = cp.tile([P, NSLOT * HALF], f32)
            for s in range(NSLOT):
                nc.sync.dma_start(
                    cosk[:, s * HALF : (s + 1) * HALF], cosk_in[s * P : (s + 1) * P, :]
                )
                nc.sync.dma_start(
                    sink[:, s * HALF : (s + 1) * HALF], sink_in[s * P : (s + 1) * P, :]
                )
            eps_t = cp.tile([P, 1], f32)
            nc.vector.memset(eps_t[:], 1e-6)
            w_all = cp.tile([P, NSLOT, NH], f32)  # mixing weights per slot
            swap_sb = cp.tile([P, ROPE], f32)
            nc.sync.dma_start(swap_sb[:], swap_in[:])
            cosqT = cp.tile([ROPE, TPC], f32)
            sinqT = cp.tile([ROPE, TPC], f32)
            nc.sync.dma_start(cosqT[0:HALF, :], cosqT_in[:])
            nc.sync.dma_start(cosqT[HALF:ROPE, :], cosqT_in[:])
            nc.sync.dma_start(sinqT[0:HALF, :], sinqT_in[:])
            nc.sync.dma_start(sinqT[HALF:ROPE, :], sinqT_in[:])

            # ---------- stage A: k & w per slot ----------
            CC_H = HSZ // P  # 56
            with (
                tc.tile_pool(name="stA", bufs=2) as pA,
                tc.tile_pool(name="stAw", bufs=1) as pAw,
                tc.tile_pool(name="psA", bufs=2, space="PSUM") as psA,
                tc.tile_pool(name="psKW", bufs=1, space="PSUM") as psKW,
            ):
                kw_sb = pAw.tile([P, CC_H, 192], f32)
                nc.sync.dma_start(
                    kw_sb[:, :, 0:HD], wk_in[:].rearrange("(a p) d -> p a d", p=P)
                )
                nc.sync.dma_start(
                    kw_sb[:, :, HD : HD + NH], wp_in[:].rearrange("(a p) d -> p a d", p=P)
                )
                kT_loc = pAw.tile([P, NSLOT, P], f32)
                for s in range(NSLOT):
                    hid_t = pA.tile([P, HSZ], f32, tag="hid")
                    nc.sync.dma_start(hid_t[:], hid_c[s * P : (s + 1) * P, :])
                    pkw = psKW.tile([P, 192], f32, tag="kw")
                    for cc in range(CC_H):
                        ptr = psA.tile([P, P], f32, tag="tr")
                        nc.tensor.transpose(
                            ptr[:], hid_t[:, cc * P : (cc + 1) * P], ident[:]
                        )
                        hT = pA.tile([P, P], f32, tag="hT")
                        nc.scalar.copy(hT[:], ptr[:])
                        nc.tensor.matmul(
                            pkw[:], hT[:], kw_sb[:, cc, :],
                            start=(cc == 0), stop=(cc == CC_H - 1),
                        )
                    # layernorm on k part
                    mu = pA.tile([P, 1], f32, tag="mu")
                    nc.vector.tensor_reduce(mu[:], pkw[:, :HD], mybir.AxisListType.X, OP.add)
                    nc.vector.tensor_scalar_mul(mu[:], mu[:], 1.0 / HD)
                    d_sb = pA.tile([P, HD], f32, tag="d")
                    nc.vector.tensor_scalar(d_sb[:], pkw[:, :HD], mu[:], None, OP.subtract)
                    sq = pA.tile([P, HD], f32, tag="sq")
                    nc.vector.tensor_tensor(sq[:], d_sb[:], d_sb[:], OP.mult)
                    var = pA.tile([P, 1], f32, tag="var")
                    nc.vector.tensor_reduce(var[:], sq[:], mybir.AxisListType.X, OP.add)
                    std = pA.tile([P, 1], f32, tag="std")
                    nc.scalar.activation(std[:], var[:], AF.Sqrt, bias=eps_t[:], scale=1.0 / HD)
                    rstd = pA.tile([P, 1], f32, tag="rstd")
                    nc.vector.reciprocal(rstd[:], std[:])
                    kn = pA.tile([P, HD], f32, tag="kn")
                    nc.vector.scalar_tensor_tensor(
                        kn[:], d_sb[:], rstd[:], kgb[:, :HD],
                        op0=OP.mult, op1=OP.mult,
                    )
                    nc.vector.tensor_tensor(
                        kn[:], kn[:], kgb[:, HD:], OP.add
                    )
                    # rope on k (first 64 dims), result into kf
                    kf = pA.tile([P, HD], f32, tag="kf")
                    cs = cosk[:, s * HALF : (s + 1) * HALF]
                    sn = sink[:, s * HALF : (s + 1) * HALF]
                    x1 = kn[:, 0:HALF]
                    x2 = kn[:, HALF:ROPE]
                    t1 = pA.tile([P, HALF], f32, tag="t1")
                    nc.vector.tensor_tensor(kf[:, 0:HALF], x1, cs, OP.mult)
                    nc.vector.tensor_tensor(t1[:], x2, sn, OP.mult)
                    nc.vector.tensor_tensor(kf[:, 0:HALF], kf[:, 0:HALF], t1[:], OP.subtract)
                    nc.vector.tensor_tensor(kf[:, HALF:ROPE], x1, sn, OP.mult)
                    nc.vector.tensor_tensor(t1[:], x2, cs, OP.mult)
                    nc.vector.tensor_tensor(kf[:, HALF:ROPE], kf[:, HALF:ROPE], t1[:], OP.add)
                    nc.vector.tensor_copy(kf[:, ROPE:], kn[:, ROPE:])
                    # w for this slot (scaled)
                    nc.scalar.activation(w_all[:, s, :], pkw[:, HD : HD + NH], AF.Copy, scale=wscale)
                    # transpose k -> kT and stash
                    ptr2 = psA.tile([P, P], f32, tag="tr")
                    nc.tensor.transpose(ptr2[:], kf[:], ident[:])
                    nc.scalar.copy(kT_loc[:, s, :], ptr2[:])
                    nc.sync.dma_start(ktl[s], kT_loc[:, s, :])

                # ---------- all-gather k ----------
                import os as _os
                if _os.environ.get("KERNEL_NO_CC"):
                    for _r in range(NCORES):
                        nc.sync.dma_start(ktg[_r], ktl[:])
                else:
                    nc.gpsimd.collective_compute(
                        "AllGather",
                        OP.bypass,
                        replica_groups=[list(range(NCORES))],
                        ins=[ktl[:]],
                        outs=[ktg[:]],
                    )

            with tc.tile_pool(name="ktp", bufs=1) as pkt:
                kT = pkt.tile([P, T], f32)
                # slot j of rank r holds global tile SLOT_BASE[j] + r
                for r in range(NCORES):
                    for j in range(NSLOT):
                        m = SLOT_BASE[j] + r
                        nc.sync.dma_start(kT[:, m * P : (m + 1) * P], ktg[r, j])

                # ---------- qrT (transposed qr chunks) ----------
                CC_Q = QLR // P  # 12
                with tc.tile_pool(name="qrT", bufs=1) as pqrT:
                    qrT = [pqrT.tile([P, TPC], f32, name=f"qrT{cc}", tag=f"qrT{cc}") for cc in range(CC_Q)]
                    with (
                        tc.tile_pool(name="qrl", bufs=2) as pqr,
                        tc.tile_pool(name="psB", bufs=2, space="PSUM") as psB,
                    ):
                        for s in range(NSLOT):
                            qr_t = pqr.tile([P, QLR], f32, tag="qr")
                            nc.sync.dma_start(qr_t[:], qr_c[s * P : (s + 1) * P, :])
                            for cc in range(CC_Q):
                                ptr = psB.tile([P, P], f32, tag="tr")
                                nc.tensor.transpose(
                                    ptr[:], qr_t[:, cc * P : (cc + 1) * P], ident[:]
                                )
                                nc.scalar.copy(qrT[cc][:, s * P : (s + 1) * P], ptr[:])

                    # ---------- main loop: slot-outer so each slot's top-k
                    # extraction (DVE) overlaps later slots' scores (PE/ACT).
                    # Costs a 4x re-stream of wq_b, which hides under the
                    # extraction phase's idle DMA.
                    with (
                        tc.tile_pool(name="wqp", bufs=3) as pwq,
                        tc.tile_pool(name="qtp", bufs=2) as pqt,
                        tc.tile_pool(name="accp", bufs=1) as pacc,
                        tc.tile_pool(name="rp", bufs=3) as prp,
                        tc.tile_pool(name="mk", bufs=2) as pmk,
                        tc.tile_pool(name="ix", bufs=2) as pix,
                        tc.tile_pool(name="psQ", bufs=1, space="PSUM") as psQ,
                        tc.tile_pool(name="psZ", bufs=2, space="PSUM") as psZ,
                    ):
                        acc = [pacc.tile([P, SLOT_W[s]], f32, name=f"acc{s}", tag=f"acc{s}") for s in range(NSLOT)]
                        GH = 4  # heads per group
                        for s in range(NSLOT):
                            sl = slice(s * P, (s + 1) * P)
                            for hg in range(NH // GH):
                                psq = [psQ.tile([P, P], f32, name=f"psq{i}", tag=f"psq{i}") for i in range(GH)]
                                for cc in range(CC_Q):
                                    wqt = pwq.tile([P, GH * HD], f32, tag="wq")
                                    nc.sync.dma_start(
                                        wqt[:],
                                        wq_b[cc * P : (cc + 1) * P,
                                             hg * GH * HD : (hg + 1) * GH * HD],
                                    )
                                    for i in range(GH):
                                        nc.tensor.matmul(
                                            psq[i][:], wqt[:, i * HD : (i + 1) * HD],
                                            qrT[cc][:, sl],
                                            start=(cc == 0), stop=(cc == CC_Q - 1),
                                        )
                                qts = []
                                for i in range(GH):
                                    qt = pqt.tile([P, P], f32, tag=f"qt{i}")
                                    qts.append(qt)
                                    qsb = prp.tile([P, P], f32, tag="qsb")
                                    nc.scalar.activation(qsb[:], psq[i][:], AF.Copy, scale=qscale)
                                    nc.scalar.copy(qt[ROPE:, :], qsb[ROPE:, :])
                                    # swap rotary halves across partitions via PE:
                                    # psw[0:32] = x2, psw[32:64] = x1
                                    psw = psZ.tile([ROPE, P], f32, tag="psw")
                                    nc.tensor.matmul(psw[:], swap_sb[:], qsb[:], start=True, stop=True)
                                    # rot1 = x1*cos - x2*sin  (partitions 0:32)
                                    t1 = prp.tile([ROPE, P], f32, tag="ropet")
                                    nc.vector.tensor_tensor(t1[0:HALF, :], psw[0:HALF, :], sinqT[0:HALF, sl], OP.mult)
                                    nc.vector.tensor_tensor(qt[0:HALF, :], qsb[0:HALF, :], cosqT[0:HALF, sl], OP.mult)
                                    nc.vector.tensor_tensor(qt[0:HALF, :], qt[0:HALF, :], t1[0:HALF, :], OP.subtract)
                                    # rot2 = x1*sin + x2*cos  (partitions 32:64)
                                    nc.vector.tensor_tensor(t1[HALF:ROPE, :], psw[HALF:ROPE, :], sinqT[HALF:ROPE, sl], OP.mult)
                                    nc.vector.tensor_tensor(qt[HALF:ROPE, :], qsb[HALF:ROPE, :], cosqT[HALF:ROPE, sl], OP.mult)
                                    nc.vector.tensor_tensor(qt[HALF:ROPE, :], qt[HALF:ROPE, :], t1[HALF:ROPE, :], OP.add)
                                # scores for this slot
                                for j in range(SLOT_NJ[s]):
                                    for i in range(GH):
                                        h = hg * GH + i
                                        z = psZ.tile([P, 512], f32, tag="z")
                                        nc.tensor.matmul(
                                            z[:], qts[i][:],
                                            kT[:, j * 512 : (j + 1) * 512],
                                            start=True, stop=True,
                                        )
                                        r = prp.tile([P, 512], f32, tag="r")
                                        nc.scalar.activation(r[:], z[:], AF.Relu)
                                        a_sl = acc[s][:, j * 512 : (j + 1) * 512]
                                        if h == 0:
                                            nc.vector.tensor_scalar(
                                                a_sl, r[:], w_all[:, s, h : h + 1], None, OP.mult
                                            )
                                        else:
                                            nc.vector.scalar_tensor_tensor(
                                                a_sl, r[:], w_all[:, s, h : h + 1], a_sl,
                                                op0=OP.mult, op1=OP.add,
                                            )

                            # ---------- mask + scores out + topk for slot s ----------
                            tcol = posc[:, s : s + 1]
                            for j in range(SLOT_NJ[s]):
                                mask = pmk.tile([P, 512], i32, tag="mask")
                                nc.vector.tensor_scalar(
                                    mask[:],
                                    spos[:, j * 512 : (j + 1) * 512],
                                    tcol, None, OP.is_gt,
                                )
                                scrat = pmk.tile([P, 512], f32, tag="scrat")
                                nc.scalar.copy(scrat[:], acc[s][:, j * 512 : (j + 1) * 512])
                                nc.vector.copy_predicated(scrat[:], mask[:], neginf[:])
                                nc.sync.dma_start(
                                    scores_c[s * P : (s + 1) * P, j * 512 : (j + 1) * 512],
                                    scrat[:],
                                )
                                nc.vector.copy_predicated(
                                    acc[s][:, j * 512 : (j + 1) * 512], mask[:],
                                    negfill[:],
                                )
                            for j in range(SLOT_NJ[s], 8):
                                nc.sync.dma_start(
                                    scores_c[s * P : (s + 1) * P, j * 512 : (j + 1) * 512],
                                    neginf[:],
                                )
                            # ---- extraction ----
                            idxt = pix.tile([P, TOPK], u32, tag="idx")
                            mx = pmk.tile([P, 8], f32, tag="mx")
                            for rnd in range(SLOT_R[s]):
                                nc.vector.max(out=mx[:], in_=acc[s][:])
                                nc.vector.max_index(
                                    out=idxt[:, rnd * 8 : rnd * 8 + 8],
                                    in_max=mx[:], in_values=acc[s][:],
                                )
                                if rnd < SLOT_R[s] - 1:
                                    nc.vector.match_replace(
                                        out=acc[s][:], in_to_replace=mx[:],
                                        in_values=acc[s][:], imm_value=NEG_FILL,
                                    )
                            # deterministic tail: idx[t, r] = r for r > t
                            mt = pmk.tile([P, TOPK], i32, tag="mtail")
                            nc.vector.tensor_scalar(
                                mt[:], iota_f[:], tcol, None, OP.is_gt
                            )
                            nc.vector.copy_predicated(
                                idxt[:], mt[:], iota_u[:]
                            )
                            nc.sync.dma_start(
                                idx_c[s * P : (s + 1) * P, :], idxt[:].bitcast(i32)
                            )

    nc.compile()
    return nc


def _get_compiled():
    global _compiled
    if _compiled is None:
        _compiled = _build()
    return _compiled


def kernel(hidden_states, qr, positions, wq_b, wk, k_gamma, k_beta, w_proj):
    from concourse import bass_utils

    hidden_states = np.asarray(hidden_states, dtype=np.float32)
    qr = np.asarray(qr, dtype=np.float32)
    positions = np.asarray(positions, dtype=np.int32)
    wq_b = np.asarray(wq_b, dtype=np.float32)
    wk = np.asarray(wk, dtype=np.float32)
    k_gamma = np.asarray(k_gamma, dtype=np.float32).reshape(1, HD)
    k_beta = np.asarray(k_beta, dtype=np.float32).reshape(1, HD)
    w_proj = np.asarray(w_proj, dtype=np.float32)

    nc = _get_compiled()

    # rope tables (host, fp32, mirrors reference numerics)
    inv_freq = (1.0 / (np.float32(10000.0) ** (np.arange(HALF, dtype=np.float32) / np.float32(HALF)))).astype(np.float32)
    posf = positions.astype(np.float32)
    ang = posf[:, None] * inv_freq[None, :]
    cosa = np.cos(ang).astype(np.float32)
    sina = np.sin(ang).astype(np.float32)
    qs = np.float32(HD ** -0.5)

    ident = np.eye(P, dtype=np.float32)
    swap = np.zeros((P, ROPE), dtype=np.float32)
    for m in range(ROPE):
        swap[(m + HALF) % ROPE, m] = 1.0
    iota = np.arange(TOPK, dtype=np.uint32).reshape(1, TOPK)
    sposf = posf.reshape(1, T)

    in_maps = []
    row_maps = []
    for c in range(NCORES):
        rows = np.concatenate(
            [np.arange(128 * (SLOT_BASE[s] + c), 128 * (SLOT_BASE[s] + c) + 128) for s in range(NSLOT)]
        )
        row_maps.append(rows)
        in_maps.append(
            dict(
                qr_c=np.ascontiguousarray(qr[rows]),
                hid_c=np.ascontiguousarray(hidden_states[rows]),
                wq_b=wq_b,
                wk_in=wk,
                wp_in=w_proj,
                kg_in=k_gamma,
                kb_in=k_beta,
                posc_in=np.ascontiguousarray(posf[rows].reshape(TPC, 1)),
                spos_in=sposf,
                cosk_in=np.ascontiguousarray(cosa[rows]),
                sink_in=np.ascontiguousarray(sina[rows]),
                cosqT_in=np.ascontiguousarray(cosa[rows].T),
                sinqT_in=np.ascontiguousarray(sina[rows].T),
                ident_in=ident,
                swap_in=swap,
                iota_in=iota,
            )
        )

    import os as _os
    trace = bool(_os.environ.get("KERNEL_TRACE"))
    res = bass_utils.run_bass_kernel_spmd(
        nc, in_maps, core_ids=list(range(NCORES)), trace=trace
    )
    global last_result
    last_result = res

    scores = np.empty((T, T), dtype=np.float32)
    topk_idx = np.empty((T, TOPK), dtype=np.int32)
    for c in range(NCORES):
        scores[row_maps[c]] = res.results[c]["scores_c"]
        topk_idx[row_maps[c]] = res.results[c]["idx_c"]
    return topk_idx, scores


# revision 19
# speedup vs baseline: 1.1064x; 1.1064x over previous
#
# DeepseekV4 Indexer kernel for 8x Trainium2 NeuronCores (Bass/Tile).
#
# Sharding: data-parallel over query-token tiles, interleaved so every core
# gets one tile from each causal-width band: core c owns global 128-row tiles
# {24+c, 16+c, 8+c, c} (slots 0..3). k is computed locally per-core for its
# own rows and all-gathered (transposed) across the 8 cores.
#
# Per (t,s) score tile: per head h, PE computes Z_h = q_hT.T @ kT (fp32),
# ACT evicts relu(Z_h) to SBUF, DVE accumulates acc += relu(Z_h) * w[t,h]
# via scalar_tensor_tensor (exact fp32, same head order as the reference
# scan). Causal masking is data-driven from the positions input so the
# program is SPMD-uniform. Top-k = iterative max8/max_index/match_replace
# extraction (exact, jax-stable tie order), with the deterministic
# "masked tail" of each row (output[t, r] = r for r > t) filled by a
# predicated iota overwrite instead of being extracted.
#
import sys
import math

sys.path.insert(0, "/opt/trn_rl_repo")

import numpy as np

T = 4096
HSZ = 7168
QLR = 1536
NH = 64
HD = 128
ROPE = 64
HALF = ROPE // 2  # 32
TOPK = 2048
NCORES = 8
P = 128

SLOT_BASE = [24, 16, 8, 0]          # slot s of core c handles global tile SLOT_BASE[s]+c
SLOT_W = [4096, 3072, 2048, 1024]   # uniform extraction/score width per slot
SLOT_NJ = [8, 6, 4, 2]              # number of 512-wide s-tiles computed per slot
SLOT_R = [256, 256, 256, 128]       # extraction rounds per slot
NSLOT = 4
TPC = NSLOT * P                      # tokens per core = 512

NEG_FILL = -3.0e38

_compiled = None
last_result = None


def _build():
    import concourse.bacc as bacc
    import concourse.mybir as mybir
    from concourse.tile import TileContext

    f32 = mybir.dt.float32
    u32 = mybir.dt.uint32
    i32 = mybir.dt.int32
    AF = mybir.ActivationFunctionType
    OP = mybir.AluOpType

    nc = bacc.Bacc("TRN2", target_bir_lowering=False, debug=False, num_devices=NCORES)

    # ---- per-core I/O ----
    qr_c = nc.dram_tensor("qr_c", [TPC, QLR], f32, kind="ExternalInput")
    hid_c = nc.dram_tensor("hid_c", [TPC, HSZ], f32, kind="ExternalInput")
    wq_b = nc.dram_tensor("wq_b", [QLR, NH * HD], f32, kind="ExternalInput")
    wk_in = nc.dram_tensor("wk_in", [HSZ, HD], f32, kind="ExternalInput")
    wp_in = nc.dram_tensor("wp_in", [HSZ, NH], f32, kind="ExternalInput")
    kg_in = nc.dram_tensor("kg_in", [1, HD], f32, kind="ExternalInput")
    kb_in = nc.dram_tensor("kb_in", [1, HD], f32, kind="ExternalInput")
    posc_in = nc.dram_tensor("posc_in", [TPC, 1], f32, kind="ExternalInput")
    spos_in = nc.dram_tensor("spos_in", [1, T], f32, kind="ExternalInput")
    cosk_in = nc.dram_tensor("cosk_in", [TPC, HALF], f32, kind="ExternalInput")
    sink_in = nc.dram_tensor("sink_in", [TPC, HALF], f32, kind="ExternalInput")
    # transposed + HD^-0.5-scaled rope tables for q (freq-major, core tokens)
    cosqT_in = nc.dram_tensor("cosqT_in", [HALF, TPC], f32, kind="ExternalInput")
    sinqT_in = nc.dram_tensor("sinqT_in", [HALF, TPC], f32, kind="ExternalInput")
    ident_in = nc.dram_tensor("ident_in", [P, P], f32, kind="ExternalInput")
    swap_in = nc.dram_tensor("swap_in", [P, ROPE], f32, kind="ExternalInput")
    iota_in = nc.dram_tensor("iota_in", [1, TOPK], u32, kind="ExternalInput")

    scores_c = nc.dram_tensor("scores_c", [TPC, T], f32, kind="ExternalOutput")
    idx_c = nc.dram_tensor("idx_c", [TPC, TOPK], i32, kind="ExternalOutput")

    # collectives buffers
    ktl = nc.dram_tensor("ktl", [NSLOT, P, P], f32)
    ktg = nc.dram_tensor("ktg", [NCORES, NSLOT, P, P], f32, addr_space="Shared")

    qscale = float(HD ** -0.5)
    wscale = float(NH ** -0.5)

    with TileContext(nc) as tc:
        with tc.tile_pool(name="const", bufs=1) as cp:
            ident = cp.tile([P, P], f32)
            nc.sync.dma_start(ident[:], ident_in[:])
            neginf = cp.tile([P, 512], f32)
            nc.vector.memset(neginf[:], float("-inf"))
            negfill = cp.tile([P, 512], f32)
            nc.vector.memset(negfill[:], NEG_FILL)
            iota_u = cp.tile([P, TOPK], u32)
            nc.sync.dma_start(iota_u[:], iota_in[:].partition_broadcast(P))
            iota_f = cp.tile([P, TOPK], f32)
            nc.vector.tensor_copy(iota_f[:], iota_u[:])
            spos = cp.tile([P, T], f32)
            nc.sync.dma_start(spos[:], spos_in[:].partition_broadcast(P))
            posc = cp.tile([P, NSLOT], f32)
            for s in range(NSLOT):
                nc.sync.dma_start(posc[:, s : s + 1], posc_in[s * P : (s + 1) * P, :])
            kgb = cp.tile([P, 2 * HD], f32)
            nc.sync.dma_start(kgb[:, :HD], kg_in[:].partition_broadcast(P))
            nc.sync.dma_start(kgb[:, HD:], kb_in[:].partition_broadcast(P))
            cosk = cp.tile([P, NSLOT * HALF], f32)
            sink = cp.tile([P, NSLOT * HALF], f32)
            for s in range(NSLOT):
                nc.sync.dma_start(
                    cosk[:, s * HALF : (s + 1) * HALF], cosk_in[s * P : (s + 1) * P, :]
                )
                nc.sync.dma_start(
                    sink[:, s * HALF : (s + 1) * HALF], sink_in[s * P : (s + 1) * P, :]
                )
            eps_t = cp.tile([P, 1], f32)
            nc.vector.memset(eps_t[:], 1e-6)
            w_all = cp.tile([P, NSLOT, NH], f32)  # mixing weights per slot
            swap_sb = cp.tile([P, ROPE], f32)
            nc.sync.dma_start(swap_sb[:], swap_in[:])
            cosqT = cp.tile([ROPE, TPC], f32)
            sinqT = cp.tile([ROPE, TPC], f32)
            nc.sync.dma_start(cosqT[0:HALF, :], cosqT_in[:])
            nc.sync.dma_start(cosqT[HALF:ROPE, :], cosqT_in[:])
            nc.sync.dma_start(sinqT[0:HALF, :], sinqT_in[:])
            nc.sync.dma_start(sinqT[HALF:ROPE, :], sinqT_in[:])

            # ---------- stage A: k & w per slot ----------
            CC_H = HSZ // P  # 56
            with (
                tc.tile_pool(name="stA", bufs=2) as pA,
                tc.tile_pool(name="stAw", bufs=1) as pAw,
                tc.tile_pool(name="psA", bufs=2, space="PSUM") as psA,
                tc.tile_pool(name="psKW", bufs=1, space="PSUM") as psKW,
            ):
                kw_sb = pAw.tile([P, CC_H, 192], f32)
                nc.sync.dma_start(
                    kw_sb[:, :, 0:HD], wk_in[:].rearrange("(a p) d -> p a d", p=P)
                )
                nc.sync.dma_start(
                    kw_sb[:, :, HD : HD + NH], wp_in[:].rearrange("(a p) d -> p a d", p=P)
                )
                kT_loc = pAw.tile([P, NSLOT, P], f32)
                for s in range(NSLOT):
                    hid_t = pA.tile([P, HSZ], f32, tag="hid")
                    nc.sync.dma_start(hid_t[:], hid_c[s * P : (s + 1) * P, :])
                    pkw = psKW.tile([P, 192], f32, tag="kw")
                    for cc in range(CC_H):
                        ptr = psA.tile([P, P], f32, tag="tr")
                        nc.tensor.transpose(
                            ptr[:], hid_t[:, cc * P : (cc + 1) * P], ident[:]
                        )
                        hT = pA.tile([P, P], f32, tag="hT")
                        nc.scalar.copy(hT[:], ptr[:])
                        nc.tensor.matmul(
                            pkw[:], hT[:], kw_sb[:, cc, :],
                            start=(cc == 0), stop=(cc == CC_H - 1),
                        )
                    # layernorm on k part
                    mu = pA.tile([P, 1], f32, tag="mu")
                    nc.vector.tensor_reduce(mu[:], pkw[:, :HD], mybir.AxisListType.X, OP.add)
                    nc.vector.tensor_scalar_mul(mu[:], mu[:], 1.0 / HD)
                    d_sb = pA.tile([P, HD], f32, tag="d")
                    nc.vector.tensor_scalar(d_sb[:], pkw[:, :HD], mu[:], None, OP.subtract)
                    sq = pA.tile([P, HD], f32, tag="sq")
                    nc.vector.tensor_tensor(sq[:], d_sb[:], d_sb[:], OP.mult)
                    var = pA.tile([P, 1], f32, tag="var")
                    nc.vector.tensor_reduce(var[:], sq[:], mybir.AxisListType.X, OP.add)
                    std = pA.tile([P, 1], f32, tag="std")
                    nc.scalar.activation(std[:], var[:], AF.Sqrt, bias=eps_t[:], scale=1.0 / HD)
                    rstd = pA.tile([P, 1], f32, tag="rstd")
                    nc.vector.reciprocal(rstd[:], std[:])
                    kn = pA.tile([P, HD], f32, tag="kn")
                    nc.vector.scalar_tensor_tensor(
                        kn[:], d_sb[:], rstd[:], kgb[:, :HD],
                        op0=OP.mult, op1=OP.mult,
                    )
                    nc.vector.tensor_tensor(
                        kn[:], kn[:], kgb[:, HD:], OP.add
                    )
                    # rope on k (first 64 dims), result into kf
                    kf = pA.tile([P, HD], f32, tag="kf")
                    cs = cosk[:, s * HALF : (s + 1) * HALF]
                    sn = sink[:, s * HALF : (s + 1) * HALF]
                    x1 = kn[:, 0:HALF]
                    x2 = kn[:, HALF:ROPE]
                    t1 = pA.tile([P, HALF], f32, tag="t1")
                    nc.vector.tensor_tensor(kf[:, 0:HALF], x1, cs, OP.mult)
                    nc.vector.tensor_tensor(t1[:], x2, sn, OP.mult)
                    nc.vector.tensor_tensor(kf[:, 0:HALF], kf[:, 0:HALF], t1[:], OP.subtract)
                    nc.vector.tensor_tensor(kf[:, HALF:ROPE], x1, sn, OP.mult)
                    nc.vector.tensor_tensor(t1[:], x2, cs, OP.mult)
                    nc.vector.tensor_tensor(kf[:, HALF:ROPE], kf[:, HALF:ROPE], t1[:], OP.add)
                    nc.vector.tensor_copy(kf[:, ROPE:], kn[:, ROPE:])
                    # w for this slot (scaled)
                    nc.scalar.activation(w_all[:, s, :], pkw[:, HD : HD + NH], AF.Copy, scale=wscale)
                    # transpose k -> kT and stash
                    ptr2 = psA.tile([P, P], f32, tag="tr")
                    nc.tensor.transpose(ptr2[:], kf[:], ident[:])
                    nc.scalar.copy(kT_loc[:, s, :], ptr2[:])
                    nc.sync.dma_start(ktl[s], kT_loc[:, s, :])

                # ---------- all-gather k ----------
                import os as _os
                if _os.environ.get("KERNEL_NO_CC"):
                    for _r in range(NCORES):
                        nc.sync.dma_start(ktg[_r], ktl[:])
                else:
                    nc.gpsimd.collective_compute(
                        "AllGather",
                        OP.bypass,
                        replica_groups=[list(range(NCORES))],
                        ins=[ktl[:]],
                        outs=[ktg[:]],
                    )

            with tc.tile_pool(name="ktp", bufs=1) as pkt:
                kT = pkt.tile([P, T], f32)
                # slot j of rank r holds global tile SLOT_BASE[j] + r
                for r in range(NCORES):
                    for j in range(NSLOT):
                        m = SLOT_BASE[j] + r
                        nc.sync.dma_start(kT[:, m * P : (m + 1) * P], ktg[r, j])

                # ---------- qrT (transposed qr chunks) ----------
                CC_Q = QLR // P  # 12
                with tc.tile_pool(name="qrT", bufs=1) as pqrT:
                    qrT = [pqrT.tile([P, TPC], f32, name=f"qrT{cc}", tag=f"qrT{cc}") for cc in range(CC_Q)]
                    with (
                        tc.tile_pool(name="qrl", bufs=2) as pqr,
                        tc.tile_pool(name="psB", bufs=2, space="PSUM") as psB,
                    ):
                        for s in range(NSLOT):
                            qr_t = pqr.tile([P, QLR], f32, tag="qr")
                            nc.sync.dma_start(qr_t[:], qr_c[s * P : (s + 1) * P, :])
                            for cc in range(CC_Q):
                                ptr = psB.tile([P, P], f32, tag="tr")
                                nc.tensor.transpose(
                                    ptr[:], qr_t[:, cc * P : (cc + 1) * P], ident[:]
                                )
                                nc.scalar.copy(qrT[cc][:, s * P : (s + 1) * P], ptr[:])

                    # ---------- main loop: slot-outer so each slot's top-k
                    # extraction (DVE) overlaps later slots' scores (PE/ACT).
                    # Costs a 4x re-stream of wq_b, which hides under the
                    # extraction phase's idle DMA.
                    with (
                        tc.tile_pool(name="wqp", bufs=3) as pwq,
                        tc.tile_pool(name="qtp", bufs=2) as pqt,
                        tc.tile_pool(name="accp", bufs=1) as pacc,
                        tc.tile_pool(name="rp", bufs=3) as prp,
                        tc.tile_pool(name="mk", bufs=2) as pmk,
                        tc.tile_pool(name="ix", bufs=2) as pix,
                        tc.tile_pool(name="psQ", bufs=1, space="PSUM") as psQ,
                        tc.tile_pool(name="psZ", bufs=2, space="PSUM") as psZ,
                    ):
                        acc = [pacc.tile([P, SLOT_W[s]], f32, name=f"acc{s}", tag=f"acc{s}") for s in range(NSLOT)]
                        GH = 4  # heads per group
                        # smallest scores-work slot first: its extraction starts
                        # soonest and the PE stays ahead of the DVE chain
                        for s in (3, 2, 1, 0):
                            sl = slice(s * P, (s + 1) * P)
                            for hg in range(NH // GH):
                                psq = [psQ.tile([P, P], f32, name=f"psq{i}", tag=f"psq{i}") for i in range(GH)]
                                for cc in range(CC_Q):
                                    wqt = pwq.tile([P, GH * HD], f32, tag="wq")
                                    nc.sync.dma_start(
                                        wqt[:],
                                        wq_b[cc * P : (cc + 1) * P,
                                             hg * GH * HD : (hg + 1) * GH * HD],
                                    )
                                    for i in range(GH):
                                        nc.tensor.matmul(
                                            psq[i][:], wqt[:, i * HD : (i + 1) * HD],
                                            qrT[cc][:, sl],
                                            start=(cc == 0), stop=(cc == CC_Q - 1),
                                        )
                                qts = []
                                for i in range(GH):
                                    qt = pqt.tile([P, P], f32, tag=f"qt{i}")
                                    qts.append(qt)
                                    qsb = prp.tile([P, P], f32, tag="qsb")
                                    nc.scalar.activation(qsb[:], psq[i][:], AF.Copy, scale=qscale)
                                    nc.scalar.copy(qt[ROPE:, :], qsb[ROPE:, :])
                                    # swap rotary halves across partitions via PE:
                                    # psw[0:32] = x2, psw[32:64] = x1
                                    psw = psZ.tile([ROPE, P], f32, tag="psw")
                                    nc.tensor.matmul(psw[:], swap_sb[:], qsb[:], start=True, stop=True)
                                    # rope runs on GPSIMD: the DVE is the
                                    # saturated engine (extraction chain), and
                                    # gpsimd is otherwise idle. gpsimd cannot
                                    # read PSUM, so psw is evicted via ACT.
                                    pswb = prp.tile([ROPE, P], f32, tag="pswb")
                                    nc.scalar.copy(pswb[:], psw[:])
                                    t1 = prp.tile([ROPE, P], f32, tag="ropet")
                                    # rot1 = x1*cos - x2*sin  (partitions 0:32)
                                    nc.gpsimd.tensor_tensor(t1[0:HALF, :], pswb[0:HALF, :], sinqT[0:HALF, sl], OP.mult)
                                    nc.gpsimd.tensor_tensor(qt[0:HALF, :], qsb[0:HALF, :], cosqT[0:HALF, sl], OP.mult)
                                    nc.gpsimd.tensor_tensor(qt[0:HALF, :], qt[0:HALF, :], t1[0:HALF, :], OP.subtract)
                                    # rot2 = x1*sin + x2*cos  (partitions 32:64)
                                    nc.gpsimd.tensor_tensor(t1[HALF:ROPE, :], pswb[HALF:ROPE, :], sinqT[HALF:ROPE, sl], OP.mult)
                                    nc.gpsimd.tensor_tensor(qt[HALF:ROPE, :], qsb[HALF:ROPE, :], cosqT[HALF:ROPE, sl], OP.mult)
                                    nc.gpsimd.tensor_tensor(qt[HALF:ROPE, :], qt[HALF:ROPE, :], t1[HALF:ROPE, :], OP.add)
                                # scores for this slot
                                for j in range(SLOT_NJ[s]):
                                    for i in range(GH):
                                        h = hg * GH + i
                                        z = psZ.tile([P, 512], f32, tag="z")
                                        nc.tensor.matmul(
                                            z[:], qts[i][:],
                                            kT[:, j * 512 : (j + 1) * 512],
                                            start=True, stop=True,
                                        )
                                        r = prp.tile([P, 512], f32, tag="r")
                                        nc.scalar.activation(r[:], z[:], AF.Relu)
                                        a_sl = acc[s][:, j * 512 : (j + 1) * 512]
                                        # Head accumulation: slots 0/1 run on
                                        # GPSIMD (gpsimd only supports plain
                                        # tensor_tensor, so w*R and the add are
                                        # two ops with w free-broadcast) hidden
                                        # under the previous slot's extraction;
                                        # slots 2/3 keep the fused DVE op since
                                        # their windows are too short for gpsimd.
                                        w_col = w_all[:, s, h : h + 1]
                                        if s in (0, 1):
                                            wb = w_col.to_broadcast([P, 512])
                                            if h == 0:
                                                nc.gpsimd.tensor_tensor(a_sl, r[:], wb, OP.mult)
                                            else:
                                                rw = prp.tile([P, 512], f32, tag="rw")
                                                nc.gpsimd.tensor_tensor(rw[:], r[:], wb, OP.mult)
                                                nc.gpsimd.tensor_tensor(a_sl, a_sl, rw[:], OP.add)
                                        elif h == 0:
                                            nc.vector.tensor_scalar(
                                                a_sl, r[:], w_col, None, OP.mult
                                            )
                                        else:
                                            nc.vector.scalar_tensor_tensor(
                                                a_sl, r[:], w_col, a_sl,
                                                op0=OP.mult, op1=OP.add,
                                            )

                            # ---------- mask + scores out + topk for slot s ----------
                            tcol = posc[:, s : s + 1]
                            for j in range(SLOT_NJ[s]):
                                mask = pmk.tile([P, 512], i32, tag="mask")
                                nc.vector.tensor_scalar(
                                    mask[:],
                                    spos[:, j * 512 : (j + 1) * 512],
                                    tcol, None, OP.is_gt,
                                )
                                scrat = pmk.tile([P, 512], f32, tag="scrat")
                                nc.scalar.copy(scrat[:], acc[s][:, j * 512 : (j + 1) * 512])
                                nc.vector.copy_predicated(scrat[:], mask[:], neginf[:])
                                nc.sync.dma_start(
                                    scores_c[s * P : (s + 1) * P, j * 512 : (j + 1) * 512],
                                    scrat[:],
                                )
                                nc.vector.copy_predicated(
                                    acc[s][:, j * 512 : (j + 1) * 512], mask[:],
                                    negfill[:],
                                )
                            for j in range(SLOT_NJ[s], 8):
                                nc.sync.dma_start(
                                    scores_c[s * P : (s + 1) * P, j * 512 : (j + 1) * 512],
                                    neginf[:],
                                )
                            # ---- extraction ----
                            idxt = pix.tile([P, TOPK], u32, tag="idx")
                            mx = pmk.tile([P, 8], f32, tag="mx")
                            for rnd in range(SLOT_R[s]):
                                nc.vector.max(out=mx[:], in_=acc[s][:])
                                nc.vector.max_index(
                                    out=idxt[:, rnd * 8 : rnd * 8 + 8],
                                    in_max=mx[:], in_values=acc[s][:],
                                )
                                if rnd < SLOT_R[s] - 1:
                                    nc.vector.match_replace(
                                        out=acc[s][:], in_to_replace=mx[:],
                                        in_values=acc[s][:], imm_value=NEG_FILL,
                                    )
                            # deterministic tail: idx[t, r] = r for r > t
                            mt = pmk.tile([P, TOPK], i32, tag="mtail")
                            nc.vector.tensor_scalar(
                                mt[:], iota_f[:], tcol, None, OP.is_gt
                            )
                            nc.vector.copy_predicated(
                                idxt[:], mt[:], iota_u[:]
                            )
                            nc.sync.dma_start(
                                idx_c[s * P : (s + 1) * P, :], idxt[:].bitcast(i32)
                            )

    nc.compile()
    return nc


def _get_compiled():
    global _compiled
    if _compiled is None:
        _compiled = _build()
    return _compiled


def kernel(hidden_states, qr, positions, wq_b, wk, k_gamma, k_beta, w_proj):
    from concourse import bass_utils

    hidden_states = np.asarray(hidden_states, dtype=np.float32)
    qr = np.asarray(qr, dtype=np.float32)
    positions = np.asarray(positions, dtype=np.int32)
    wq_b = np.asarray(wq_b, dtype=np.float32)
    wk = np.asarray(wk, dtype=np.float32)
    k_gamma = np.asarray(k_gamma, dtype=np.float32).reshape(1, HD)
    k_beta = np.asarray(k_beta, dtype=np.float32).reshape(1, HD)
    w_proj = np.asarray(w_proj, dtype=np.float32)

    nc = _get_compiled()

    # rope tables (host, fp32, mirrors reference numerics)
    inv_freq = (1.0 / (np.float32(10000.0) ** (np.arange(HALF, dtype=np.float32) / np.float32(HALF)))).astype(np.float32)
    posf = positions.astype(np.float32)
    ang = posf[:, None] * inv_freq[None, :]
    cosa = np.cos(ang).astype(np.float32)
    sina = np.sin(ang).astype(np.float32)
    qs = np.float32(HD ** -0.5)

    ident = np.eye(P, dtype=np.float32)
    swap = np.zeros((P, ROPE), dtype=np.float32)
    for m in range(ROPE):
        swap[(m + HALF) % ROPE, m] = 1.0
    iota = np.arange(TOPK, dtype=np.uint32).reshape(1, TOPK)
    sposf = posf.reshape(1, T)

    in_maps = []
    row_maps = []
    for c in range(NCORES):
        rows = np.concatenate(
            [np.arange(128 * (SLOT_BASE[s] + c), 128 * (SLOT_BASE[s] + c) + 128) for s in range(NSLOT)]
        )
        row_maps.append(rows)
        in_maps.append(
            dict(
                qr_c=np.ascontiguousarray(qr[rows]),
                hid_c=np.ascontiguousarray(hidden_states[rows]),
                wq_b=wq_b,
                wk_in=wk,
                wp_in=w_proj,
                kg_in=k_gamma,
                kb_in=k_beta,
                posc_in=np.ascontiguousarray(posf[rows].reshape(TPC, 1)),
                spos_in=sposf,
                cosk_in=np.ascontiguousarray(cosa[rows]),
                sink_in=np.ascontiguousarray(sina[rows]),
                cosqT_in=np.ascontiguousarray(cosa[rows].T),
                sinqT_in=np.ascontiguousarray(sina[rows].T),
                ident_in=ident,
                swap_in=swap,
                iota_in=iota,
            )
        )

    import os as _os
    trace = bool(_os.environ.get("KERNEL_TRACE"))
    res = bass_utils.run_bass_kernel_spmd(
        nc, in_maps, core_ids=list(range(NCORES)), trace=trace
    )
    global last_result
    last_result = res

    scores = np.empty((T, T), dtype=np.float32)
    topk_idx = np.empty((T, TOPK), dtype=np.int32)
    for c in range(NCORES):
        scores[row_maps[c]] = res.results[c]["scores_c"]
        topk_idx[row_maps[c]] = res.results[c]["idx_c"]
    return topk_idx, scores
